# revision 18
# baseline (speedup 1.0000x reference)
"""Trainium2 Bass kernel for GPT2Attention with soft-threshold pruning.

Shapes: hidden_states [1, 2048, 1024], H=16 heads, head_dim=64.
Sharding: 2 heads per core across 8 cores (head parallel); c_attn columns and
c_proj rows split by head group; partial c_proj outputs summed on host.

Math per reference (no 1/sqrt(d) scaling):
    w   = q @ k^T                       (causal-masked to C=-1e4)
    w'  = C + (w - C) * sigmoid(10 w)
    a   = softmax(w', axis=-1)
    out = (a @ v) merged -> @ c_proj + b

Device-side shifted score  w'' = w' - C = (w + 1e4)*sigmoid(10w)  is exactly 0
for masked entries.  Key numerical facts:

* For any row with >=256 valid entries (every block i>=1), the row max of w''
  exceeds 9990 with overwhelming probability.  A fixed shift of 10015 keeps
  exp(w''-10015) within bf16/fp32 range and masked entries' exp(0-10015)
  underflow to exactly 0 -- matching the reference's own fp32 underflow.  No
  per-row max is needed outside block 0, which lets scores be computed
  TRANSPOSED ([k,q] tiles): A@V then needs no transposes of p, and the softmax
  denominator falls out of the A@V matmul via a ones-column appended to V.
* oT [hd, q] is normalized by 1/denom broadcast along partitions via a tiny
  fp32 matmul (ones[1,128]^T @ recip_row[1,512]).
* Block 0 (rows 0..127) uses the exact per-row max in the [q,k] orientation
  plus the masked-tail correction: denom += (S-128)*e^-m, numer += e^-m *
  suffix_sum(V), carried through the A@V path as a 17th "V block" (row 0 =
  [suffix sums | S-128]) against a transposed-p extension row e^-m/denom.
* fp16 for q/k (sigmoid transition needs accurate scores), bf16 for p/V/out
  projections (range), fp32 sigmoid/ws/denoms, block 0 fully fp32.
"""

import os
import sys

for _p in ("/opt/trn_rl_repo", "/root/.axon_site/_ro/trn_rl_repo"):
    if os.path.isdir(_p) and _p not in sys.path:
        sys.path.insert(0, _p)

import numpy as np

import concourse.bass as bass
import concourse.tile as tile
from concourse import bacc, mybir
from concourse.masks import make_identity

F32 = mybir.dt.float32
F16 = mybir.dt.float16
BF = mybir.dt.bfloat16
AF = mybir.ActivationFunctionType
ALU = mybir.AluOpType

S = 2048          # sequence length
D = 1024          # model dim
H = 16            # heads
HD = 64           # head dim
P = 128           # partitions
NB = S // P       # 16 seq blocks
NQ = 4            # q super-blocks of 512
QW = S // NQ      # 512
NCORES = 8
HPC = H // NCORES  # 2 heads per core
VW = HD + 1        # v columns per head incl ones column
CSHIFT = 10000.0   # -C
SLOPE = 10.0
SHIFT = 10015.0    # fixed softmax shift for blocks >= 1

_CACHE = {}


def _build_nc():
    nc = bacc.Bacc(None, target_bir_lowering=False)

    hst_d = nc.dram_tensor("hst", [P, (D // P) * S], F16, kind="ExternalInput")
    wqkv_d = nc.dram_tensor("wqkv", [P, (D // P) * 3 * P], F16, kind="ExternalInput")
    bq_d = nc.dram_tensor("bq", [P, 1], F32, kind="ExternalInput")
    bk_d = nc.dram_tensor("bk", [P, 1], F32, kind="ExternalInput")
    bv_d = nc.dram_tensor("bv", [P, 1], F32, kind="ExternalInput")
    wp_d = nc.dram_tensor("wp", [P, D], BF, kind="ExternalInput")
    out_d = nc.dram_tensor("out", [S, D], F16, kind="ExternalOutput")

    with tile.TileContext(nc) as tc:
        with (
            tc.tile_pool(name="const", bufs=1) as cpool,
            tc.tile_pool(name="pers", bufs=1) as pers,
        ):
            identB = cpool.tile([P, P], BF)
            make_identity(nc, identB)
            onesB = cpool.tile([P, 1], BF)
            nc.vector.memset(onesB, 1.0)
            ones_row = cpool.tile([1, P], F32)
            nc.vector.memset(ones_row, 1.0)
            bq_sb = cpool.tile([P, 1], F32)
            nc.sync.dma_start(bq_sb, bq_d[:])
            bk_sb = cpool.tile([P, 1], F32)
            nc.sync.dma_start(bk_sb, bk_d[:])
            bv_sb = cpool.tile([P, 1], F32)
            nc.sync.dma_start(bv_sb, bv_d[:])
            wp_sb = cpool.tile([P, D], BF)
            nc.sync.dma_start(wp_sb, wp_d[:])

            # persistent per-core tensors
            qt = pers.tile([P, S], F16)      # [hd(2 heads packed), s]
            kt = pers.tile([P, S], F16)
            qt0 = pers.tile([P, P], F32)     # fp32 copies for block 0
            kt0 = pers.tile([P, P], F32)
            # V: [k-part, blk, 2*(HD+1)] with a ones col per head; blk 16 =
            # block-0 masked-tail correction (row 0 = [suffix sums | S-128])
            v_sb = pers.tile([P, NB + 1, 2 * VW], BF)

            # ---- Phase B: QKV projections (hsT supplied pre-transposed) ----
            with (
                tc.tile_pool(name="hsload", bufs=1) as hlpool,
                tc.tile_pool(name="psB", bufs=1, space="PSUM") as psB,
            ):
                w_sb = hlpool.tile([P, D // P, 3 * P], F16)
                nc.sync.dma_start(w_sb, wqkv_d.rearrange("p (o f) -> p o f", f=3 * P))
                hsT = hlpool.tile([P, D // P, S], F16)
                for dc in range(D // P):
                    nc.sync.dma_start(hsT[:, dc, :], hst_d[:, S * dc : S * (dc + 1)])
                vT_sb = hlpool.tile([P, S], BF)

                for which, off, b_ap, dst in (
                    ("q", 0, bq_sb, qt),
                    ("k", P, bk_sb, kt),
                    ("v", 2 * P, bv_sb, vT_sb),
                ):
                    ps4 = [
                        psB.tile([P, 512], F32, tag=f"pb{sc}", name=f"ps_{which}{sc}")
                        for sc in range(4)
                    ]
                    for dc in range(D // P):
                        for sc in range(4):
                            nc.tensor.matmul(
                                ps4[sc],
                                lhsT=w_sb[:, dc, off : off + P],
                                rhs=hsT[:, dc, 512 * sc : 512 * (sc + 1)],
                                start=(dc == 0),
                                stop=(dc == D // P - 1),
                            )
                    for sc in range(4):
                        nc.scalar.activation(
                            dst[:, 512 * sc : 512 * (sc + 1)],
                            ps4[sc],
                            AF.Identity,
                            bias=b_ap,
                        )
                    if which == "q":
                        nc.vector.tensor_scalar(
                            qt0, ps4[0][:, 0:P], bq_sb, None, ALU.add
                        )
                    elif which == "k":
                        nc.vector.tensor_scalar(
                            kt0, ps4[0][:, 0:P], bk_sb, None, ALU.add
                        )

                # V -> [k-part, blk, col] via PE transposes of vT
                for grp in range(4):
                    st = psB.tile([P, 512], BF, tag="vst", name=f"vst{grp}")
                    for j4 in range(4):
                        j = grp * 4 + j4
                        nc.tensor.transpose(
                            st[:, P * j4 : P * (j4 + 1)],
                            vT_sb[:, P * j : P * (j + 1)],
                            identB,
                        )
                    st4 = st.rearrange("p (b f) -> p b f", b=4)
                    nc.vector.tensor_copy(
                        v_sb[:, grp * 4 : grp * 4 + 4, 0:HD], st4[:, :, 0:HD]
                    )
                    nc.vector.tensor_copy(
                        v_sb[:, grp * 4 : grp * 4 + 4, VW : VW + HD],
                        st4[:, :, HD : 2 * HD],
                    )
                # ones columns for the softmax denominator
                nc.vector.memset(v_sb[:, 0:NB, HD : HD + 1], 1.0)
                nc.vector.memset(v_sb[:, 0:NB, VW + HD : VW + HD + 1], 1.0)

                # suffix-V sums (block-0 masked-tail correction)
                bs = psB.tile([P, NB], F32, tag="bsum")
                for j in range(1, NB):
                    for h in range(HPC):
                        nc.tensor.matmul(
                            bs[HD * h : HD * (h + 1), j : j + 1],
                            lhsT=v_sb[:, j, VW * h : VW * h + HD],
                            rhs=onesB,
                            start=True,
                            stop=True,
                        )
                ssuf0 = hlpool.tile([P, 1], F32)
                nc.vector.tensor_reduce(
                    ssuf0, bs[:, 1:NB], mybir.AxisListType.X, ALU.add
                )
                nc.vector.memset(v_sb[:, NB, :], 0.0)
                spad = hlpool.tile([P, P], BF)
                nc.vector.memset(spad, 0.0)
                nc.vector.tensor_copy(spad[:, 0:1], ssuf0)
                sufT = psB.tile([P, 512], BF, tag="vst", name="sufT")
                nc.tensor.transpose(sufT[:, 0:P], spad, identB)
                nc.vector.tensor_copy(v_sb[0:1, NB, 0:HD], sufT[0:1, 0:HD])
                nc.vector.tensor_copy(
                    v_sb[0:1, NB, VW : VW + HD], sufT[0:1, HD : 2 * HD]
                )
                nc.vector.memset(v_sb[0:1, NB, HD : HD + 1], float(S - P))
                nc.vector.memset(
                    v_sb[0:1, NB, VW + HD : VW + HD + 1], float(S - P)
                )

            # ---- Phase C: attention + projection (scores kept transposed) ----
            with (
                tc.tile_pool(name="wsq", bufs=1) as wsq,
                tc.tile_pool(name="peq", bufs=1) as peq,
                tc.tile_pool(name="osb", bufs=2) as osb,
                tc.tile_pool(name="stats", bufs=4) as stpool,
                tc.tile_pool(name="psw", bufs=2, space="PSUM") as ps_w,
                tc.tile_pool(name="psot", bufs=1, space="PSUM") as ps_ot,
                tc.tile_pool(name="psx", bufs=1, space="PSUM") as ps_x,
                tc.tile_pool(name="psy", bufs=2, space="PSUM") as ps_y,
            ):
                sig_gate = None
                zero_g = None
                for sq in range(NQ):
                    nj = 4 * (sq + 1)
                    qsl = slice(QW * sq, QW * (sq + 1))
                    ws_h = {}
                    ws0_h = {}
                    negm_h = {}
                    # --- sigmoid subphase ---
                    for h in range(HPC):
                        hp = slice(HD * h, HD * (h + 1))
                        ws = wsq.tile(
                            [P, NB * QW], F32, tag=f"ws{h}", name=f"ws{h}"
                        )
                        ws_h[h] = ws
                        for jj in range(0, nj, 2):
                            pw = ps_w.tile([P, 1024], F32, tag="w", name="pw")
                            for dj in range(2):
                                j = jj + dj
                                nc.tensor.matmul(
                                    pw[:, 512 * dj : 512 * (dj + 1)],
                                    lhsT=kt[hp, P * j : P * (j + 1)],
                                    rhs=qt[hp, qsl],
                                    start=True,
                                    stop=True,
                                )
                            nc.scalar.activation(
                                ws[:, QW * jj : QW * (jj + 2)],
                                pw,
                                AF.Sigmoid,
                                scale=SLOPE,
                                bias=sig_gate if sig_gate is not None else 0.0,
                            )
                            for dj in range(2):
                                j = jj + dj
                                if 4 * sq <= j:
                                    # zero sigma where k > q
                                    nc.gpsimd.affine_select(
                                        out=ws[:, QW * j : QW * (j + 1)],
                                        in_=ws[:, QW * j : QW * (j + 1)],
                                        pattern=[[1, QW]],
                                        channel_multiplier=-1,
                                        base=QW * sq - P * j,
                                        compare_op=ALU.is_ge,
                                        fill=0.0,
                                    )
                            nc.vector.scalar_tensor_tensor(
                                out=ws[:, QW * jj : QW * (jj + 2)],
                                in0=pw,
                                scalar=CSHIFT,
                                in1=ws[:, QW * jj : QW * (jj + 2)],
                                op0=ALU.add,
                                op1=ALU.mult,
                            )
                        if sq == 0:
                            # block-0 exact path in [q, k] orientation, fp32
                            w0 = ps_x.tile([P, 512], F32, tag="x", name="w0")
                            nc.tensor.matmul(
                                w0[:, 0:P],
                                lhsT=qt0[hp, :],
                                rhs=kt0[hp, :],
                                start=True,
                                stop=True,
                            )
                            ws0 = stpool.tile(
                                [P, P], F32, tag=f"ws0{h}", name="ws0", bufs=1
                            )
                            nc.scalar.activation(
                                ws0,
                                w0[:, 0:P],
                                AF.Sigmoid,
                                scale=SLOPE,
                                bias=sig_gate if sig_gate is not None else 0.0,
                            )
                            nc.gpsimd.affine_select(
                                out=ws0,
                                in_=ws0,
                                pattern=[[-1, P]],
                                channel_multiplier=1,
                                base=0,
                                compare_op=ALU.is_ge,
                                fill=0.0,
                            )
                            nc.vector.scalar_tensor_tensor(
                                out=ws0,
                                in0=w0[:, 0:P],
                                scalar=CSHIFT,
                                in1=ws0,
                                op0=ALU.add,
                                op1=ALU.mult,
                            )
                            ws0_h[h] = ws0
                            m0 = stpool.tile([P, 1], F32, tag="m0", name="m0")
                            nc.vector.tensor_reduce(
                                m0, ws0, mybir.AxisListType.X, ALU.max
                            )
                            negm = stpool.tile(
                                [P, 1], F32, tag=f"ng{h}", name="negm", bufs=1
                            )
                            nc.vector.tensor_scalar_mul(negm, m0, -1.0)
                            negm_h[h] = negm

                    # gate this superblock's exps on its last sigmoid output
                    wlast = ws_h[HPC - 1]
                    nshift_g = stpool.tile(
                        [P, 1], F32, tag="nshift", name="nshift_g", bufs=2
                    )
                    nc.vector.tensor_scalar(
                        nshift_g, wlast[:, QW * nj - 1 : QW * nj], 0.0, -SHIFT,
                        ALU.mult, ALU.add,
                    )
                    if sq == 0:
                        zero_g = stpool.tile(
                            [P, 1], F32, tag="zero_g", name="zero_g", bufs=1
                        )
                        nc.vector.tensor_scalar(
                            zero_g, wlast[:, QW * nj - 1 : QW * nj], 0.0, 0.0,
                            ALU.mult, ALU.add,
                        )

                    # --- exp subphase + AV + normalize ---
                    ot_sb = osb.tile([P, QW], BF, tag="ot_sb", name="ot_sb")
                    for h in range(HPC):
                        pe = peq.tile(
                            [P, NB * QW], BF, tag=f"pe{h}", name=f"pe{h}"
                        )
                        nc.scalar.activation(
                            pe[:, : QW * nj],
                            ws_h[h][:, : QW * nj],
                            AF.Exp,
                            bias=nshift_g,
                        )
                        tailT = None
                        if sq == 0:
                            ng = stpool.tile(
                                [P, 1], F32, tag=f"ngg{h}", name="negm_gated",
                                bufs=1,
                            )
                            nc.vector.tensor_add(ng, negm_h[h], zero_g)
                            p0 = stpool.tile(
                                [P, 2 * P], BF, tag=f"p0{h}", name="p0", bufs=1
                            )
                            sm0 = stpool.tile([P, 1], F32, tag="sm0", name="sm0")
                            nc.scalar.activation(
                                p0[:, 0:P],
                                ws0_h[h],
                                AF.Exp,
                                bias=ng,
                                accum_out=sm0,
                            )
                            e_sb = stpool.tile(
                                [P, 1], F32, tag=f"e{h}", name="e_sb", bufs=1
                            )
                            nc.scalar.activation(e_sb, ng, AF.Exp)
                            tail = stpool.tile([P, 1], F32, tag="tail", name="tail")
                            nc.vector.tensor_scalar_mul(tail, e_sb, float(S - P))
                            nc.vector.tensor_add(sm0, sm0, tail)
                            recip0 = stpool.tile(
                                [P, 1], F32, tag="recip0", name="recip0"
                            )
                            nc.vector.reciprocal(recip0, sm0)
                            nc.vector.memset(p0[:, P + 1 : 2 * P], 0.0)
                            nc.vector.tensor_copy(p0[:, P : P + 1], e_sb)
                            nc.vector.tensor_scalar_mul(p0, p0, recip0)
                            # transpose p0 back to [k, q]; col P -> tail row
                            px = ps_x.tile([P, 512], BF, tag="x", name="px")
                            nc.tensor.transpose(px[:, 0:P], p0[:, 0:P], identB)
                            nc.tensor.transpose(
                                px[:, P : 2 * P], p0[:, P : 2 * P], identB
                            )
                            nc.vector.tensor_copy(pe[:, 0:P], px[:, 0:P])
                            tailT = osb.tile(
                                [P, QW], BF, tag="tailT", name="tailT", bufs=1
                            )
                            nc.vector.memset(tailT[:, P:], 0.0)
                            nc.vector.tensor_copy(tailT[:, 0:P], px[:, P : 2 * P])
                        # A @ V (+ denominator via ones col of v)
                        ot = ps_ot.tile([P, QW], F32, tag="ot", name="ot")
                        for j in range(nj):
                            nc.tensor.matmul(
                                ot[0:VW, :],
                                lhsT=v_sb[:, j, VW * h : VW * (h + 1)],
                                rhs=pe[:, QW * j : QW * (j + 1)],
                                start=(j == 0),
                                stop=(j == nj - 1 and sq != 0),
                            )
                        if sq == 0:
                            nc.tensor.matmul(
                                ot[0:VW, :],
                                lhsT=v_sb[:, NB, VW * h : VW * (h + 1)],
                                rhs=tailT,
                                start=False,
                                stop=True,
                            )
                        # normalize: recip of denom row, broadcast on gpsimd
                        rrow = stpool.tile(
                            [P, QW], F32, tag="rrow", name="rrow", bufs=2
                        )
                        nc.vector.reciprocal(rrow[0:1, :], ot[HD : HD + 1, :])
                        bc = stpool.tile([P, QW], F32, tag="bc", name="bc", bufs=2)
                        nc.gpsimd.partition_broadcast(bc, rrow[0:1, :])
                        nc.vector.tensor_tensor(
                            out=ot_sb[HD * h : HD * (h + 1), :],
                            in0=ot[0:HD, :],
                            in1=bc[0:HD, :],
                            op=ALU.mult,
                        )
                    # c_proj partials for the 4 row blocks of this superblock
                    for b in range(4):
                        i = 4 * sq + b
                        y_sb = osb.tile([P, D], F16, tag="y_sb", name="y_sb")
                        for half in range(2):
                            yp = ps_y.tile([P, 512], F32, tag="y", name="yp")
                            nc.tensor.matmul(
                                yp,
                                lhsT=ot_sb[:, P * b : P * (b + 1)],
                                rhs=wp_sb[:, 512 * half : 512 * (half + 1)],
                                start=True,
                                stop=True,
                            )
                            nc.vector.tensor_copy(
                                y_sb[:, 512 * half : 512 * (half + 1)], yp
                            )
                        nc.sync.dma_start(out_d[P * i : P * (i + 1), :], y_sb)

                    # gate next superblock's sigmoids on this one's last exp
                    sig_gate = stpool.tile(
                        [P, 1], F32, tag="sgate", name="sig_gate", bufs=2
                    )
                    nc.vector.tensor_scalar(
                        sig_gate, pe[:, QW * nj - 1 : QW * nj], 0.0, 0.0,
                        ALU.mult, ALU.add,
                    )

    nc.compile()
    return nc


def _get_nc():
    if "nc" not in _CACHE:
        _CACHE["nc"] = _build_nc()
    return _CACHE["nc"]


def kernel(hidden_states, c_attn_w, c_attn_b, c_proj_w, c_proj_b):
    from concourse.bass_utils import run_bass_kernel_spmd

    hs = np.asarray(hidden_states, np.float32).reshape(S, D)
    caw = np.asarray(c_attn_w, np.float32)
    cab = np.asarray(c_attn_b, np.float32)
    cpw = np.asarray(c_proj_w, np.float32)
    cpb = np.asarray(c_proj_b, np.float32)

    # hs^T in [p, o, s] layout: hsT[p, o, s] = hs[s, 128*o + p]
    hst = np.ascontiguousarray(
        hs.T.reshape(D // P, P, S).transpose(1, 0, 2).reshape(P, (D // P) * S)
    ).astype(np.float16)

    import ml_dtypes

    in_maps = []
    for c in range(NCORES):
        heads = [HPC * c + h for h in range(HPC)]
        qcols = [caw[:, HD * h : HD * (h + 1)] for h in heads]
        kcols = [caw[:, D + HD * h : D + HD * (h + 1)] for h in heads]
        vcols = [caw[:, 2 * D + HD * h : 2 * D + HD * (h + 1)] for h in heads]
        wqkv = np.concatenate(qcols + kcols + vcols, axis=1)  # [D, 384]
        wqkv = np.ascontiguousarray(
            wqkv.reshape(D // P, P, 3 * P)
            .transpose(1, 0, 2)
            .reshape(P, (D // P) * 3 * P)
        ).astype(np.float16)
        bq = np.concatenate([cab[HD * h : HD * (h + 1)] for h in heads])
        bk = np.concatenate([cab[D + HD * h : D + HD * (h + 1)] for h in heads])
        bv = np.concatenate([cab[2 * D + HD * h : 2 * D + HD * (h + 1)] for h in heads])
        wp = np.ascontiguousarray(cpw[P * c : P * (c + 1), :]).astype(
            ml_dtypes.bfloat16
        )
        in_maps.append(
            {
                "hst": hst,
                "wqkv": wqkv,
                "bq": np.ascontiguousarray(bq.reshape(P, 1), np.float32),
                "bk": np.ascontiguousarray(bk.reshape(P, 1), np.float32),
                "bv": np.ascontiguousarray(bv.reshape(P, 1), np.float32),
                "wp": wp,
            }
        )

    nc = _get_nc()
    res = run_bass_kernel_spmd(nc, in_maps, core_ids=list(range(NCORES)))
    out = np.zeros((S, D), np.float64)
    for c in range(NCORES):
        out += np.asarray(res.results[c]["out"], np.float64)
    out = out.astype(np.float32) + cpb[None, :].astype(np.float32)
    return out.reshape(1, S, D)
